# revision 9
# baseline (speedup 1.0000x reference)
"""APPNP GNN kernel for 8 Trainium2 NeuronCores.

Distribution: nodes (feat rows / output rows) sharded 12500/core (padded to
12544). Each core owns the edges into its dst shard. Per APPNP step the
normalized state g = h*norm is AllGathered (the halo exchange), then each core:
  - ant dma_gather pulls one f32 row per edge (edges pre-grouped on the host
    into 4 src-buckets so int16 indices address <=25088-row table windows,
    dst-sorted within bucket, packed into 128-edge chunks of whole segments),
  - a per-chunk one-hot matmul on the TensorEngine pre-accumulates the segment
    sums into 32-slot PSUM groups (bf16 operands, f32 accumulation),
  - ant dma_scatter_add lands the (unique-dst) slot rows into a per-bucket
    aggregate table (pad slots go to a dump row),
  - the update g' = 0.9*norm^2*(sum of bucket tables) + 0.1*norm*h0 is fused
    on the VectorEngine; the final step applies log_softmax instead.
The 2-layer MLP encoder runs on-device before the propagation.

Device tables use a transposed row convention r = (n%128)*98 + n//128 so every
node-table DMA is one contiguous 25KB descriptor per partition; the host
permutes feat/norm inputs and inverse-permutes the output.
"""
import sys, os
sys.path.insert(0, "/opt/trn_rl_repo")
import numpy as np
import ml_dtypes

import concourse.bass as bass
import concourse.tile as tile
from concourse import bacc, mybir
from concourse.bass_utils import run_bass_kernel_spmd
from concourse.masks import make_identity

bf16 = mybir.dt.bfloat16
f32 = mybir.dt.float32
i16 = mybir.dt.int16
Alu = mybir.AluOpType
Act = mybir.ActivationFunctionType

N = 100000
F_IN = 512
H = 256
C = 64
K_ITER = 10
ALPHA = 0.1
N_CORES = 8
SH = 12500
TT = 98
SHP = 12544             # 128*98
NTAB = SHP * N_CORES    # 100352
NBUCK = 4
BUCK = NTAB // NBUCK    # 25088
GROUP = int(os.environ.get("K_GROUP", "32"))  # chunks per gather/scatter call
SLOTS = 32
DUMP = 12543            # perm(12543) == 12543 (pad node)
UPD = 7                 # node-tile cols per update op (98 = 14*7)


def _perm(n):
    return (n % 128) * TT + n // 128


# ----------------------------------------------------------------------------
# host-side graph preprocessing
# ----------------------------------------------------------------------------

def _build_core_tables(table_row, dst_row):
    """table_row: gather-table row per edge (already permuted+core-offset).
    dst_row: permuted local dst row per edge.
    Returns per-bucket packed chunk tables."""
    per_bucket = []
    for b in range(NBUCK):
        sel = (table_row >= b * BUCK) & (table_row < (b + 1) * BUCK)
        d = dst_row[sel]
        r = (table_row[sel] - b * BUCK).astype(np.int32)
        order = np.argsort(d, kind="stable")
        d = d[order]; r = r[order]
        ne = len(d)
        chunks_b = []
        if ne:
            seg_start = np.flatnonzero(np.r_[True, d[1:] != d[:-1]])
            seg_end = np.r_[seg_start[1:], ne]
            seg_len = seg_end - seg_start
            seg_dst = d[seg_start]
            assert seg_len.max() <= 128
            cur_edges = []; cur_dst = []; cur_n = 0
            for s in range(len(seg_start)):
                L = seg_len[s]
                if cur_dst and (cur_n + L > 128 or len(cur_dst) >= SLOTS):
                    chunks_b.append((cur_edges, cur_dst))
                    cur_edges = []; cur_dst = []; cur_n = 0
                cur_edges.append((seg_start[s], seg_end[s]))
                cur_dst.append(seg_dst[s])
                cur_n += L
            if cur_dst:
                chunks_b.append((cur_edges, cur_dst))
        per_bucket.append((chunks_b, r))
    return per_bucket


def _emit_core_arrays(per_bucket, GB):
    """Emit device tables for one core padded to the global schedule GB."""
    NCH = sum(GB) * GROUP
    gidx = np.zeros((16, NCH * 8), np.int16)
    oneh = np.zeros((128, NCH, SLOTS), ml_dtypes.bfloat16)
    sidx = np.full((16, NCH * 2), DUMP, np.int16)
    ci = 0
    for b in range(NBUCK):
        chunks_b, r = per_bucket[b]
        for k in range(GB[b] * GROUP):
            if k < len(chunks_b):
                segs, dsts = chunks_b[k]
                idx16 = np.zeros(128, np.int16)
                oh = np.zeros((128, SLOTS), np.float32)
                sd = np.full(SLOTS, DUMP, np.int32)
                pos = 0
                for j, ((a, e_), dd) in enumerate(zip(segs, dsts)):
                    L = e_ - a
                    idx16[pos:pos + L] = r[a:e_]
                    oh[pos:pos + L, j] = 1.0
                    sd[j] = dd
                    pos += L
                gidx[:, ci * 8:(ci + 1) * 8] = idx16.reshape(8, 16).T
                oneh[:, ci, :] = oh.astype(ml_dtypes.bfloat16)
                sidx[:, ci * 2:(ci + 1) * 2] = sd.astype(np.int16).reshape(2, 16).T
            ci += 1
    return np.tile(gidx, (8, 1)), oneh, np.tile(sidx, (8, 1))


def _preprocess(src, dst):
    loop = np.arange(N, dtype=np.int64)
    src_sl = np.concatenate([np.asarray(src, np.int64), loop])
    dst_sl = np.concatenate([np.asarray(dst, np.int64), loop])
    deg = np.bincount(dst_sl, minlength=N).astype(np.float64)
    norm = (1.0 / np.sqrt(deg)).astype(np.float32)

    src_local = src_sl % SH
    table_row = (src_sl // SH) * SHP + (src_local % 128) * TT + src_local // 128
    dst_core = dst_sl // SH
    dl = dst_sl % SH
    dst_row = ((dl % 128) * TT + dl // 128).astype(np.int32)

    per_core = []
    for c in range(N_CORES):
        m = dst_core == c
        per_core.append(_build_core_tables(table_row[m], dst_row[m]))
    return norm, per_core


# ----------------------------------------------------------------------------
# device kernel
# ----------------------------------------------------------------------------


def _dma_gather_raw(nc, out_ap, in_ap, idxs_ap, num_idxs, elem_size, elem_step):
    """bass.dma_gather minus the elem_size%256 assert (payload 128B, stride 256B)."""
    import concourse.ap_utils as ap_utils
    from concourse.bass import exact_div
    eng = nc.gpsimd
    assert idxs_ap.dtype == mybir.dt.int16
    assert in_ap.dtype == out_ap.dtype
    assert ap_utils.ap_is_contiguous(out_ap.ap[1:])
    assert ap_utils.ap_is_contiguous(idxs_ap.ap[1:])
    assert in_ap.ap[-1][1] == out_ap.ap[-1][1] == elem_size
    assert out_ap.ap[0][1] * out_ap.ap[1][1] == num_idxs
    assert in_ap.ap[0][0] == elem_step
    stride_bytes = elem_step * mybir.dt.size(in_ap.dtype)
    stride_bytes_256 = exact_div(stride_bytes, 256)
    _in_ap = eng.lower_ap_dma(in_ap, for_custom_bir_dma=True)
    _idxs_ap = eng.lower_ap(idxs_ap)
    _out_ap = eng.lower_ap(out_ap)
    inst = eng.add_instruction(
        mybir.InstDMAGatherAnt(
            name=nc.get_next_instruction_name(),
            ins=[*_in_ap, _idxs_ap, eng.lower_val_access(eng.to_reg(num_idxs))],
            outs=[_out_ap],
            transpose=False,
            num_idxs=num_idxs,
            elem_size=elem_size,
            stride_bytes_256=stride_bytes_256,
            gen_mode=0,
            single_packet=True,
            queue_num=0,
            sbuf_tokens_per_rank=0,
            sbuf_free_dim_per_rank=0,
            sbuf_free_dim_pad_per_rank=0,
            sbuf_byte_offset=0,
        )
    )
    return inst

def _build_nc(GB, k_iter):
    NCH = sum(GB) * GROUP
    nc = bacc.Bacc("TRN2", target_bir_lowering=False, debug=False,
                   num_devices=N_CORES)
    feat = nc.dram_tensor("feat", [SHP, F_IN], f32, kind="ExternalInput")
    w1 = nc.dram_tensor("w1", [F_IN, H], f32, kind="ExternalInput")
    w2 = nc.dram_tensor("w2", [H, C], f32, kind="ExternalInput")
    gidx = nc.dram_tensor("gidx", [128, NCH * 8], i16, kind="ExternalInput")
    oneh = nc.dram_tensor("oneh", [128, NCH, SLOTS], bf16, kind="ExternalInput")
    sidx = nc.dram_tensor("sidx", [128, NCH * 2], i16, kind="ExternalInput")
    nrm1 = nc.dram_tensor("nrm1", [SHP, 1], f32, kind="ExternalInput")  # alpha*norm
    sc1 = nc.dram_tensor("sc1", [SHP, 1], f32, kind="ExternalInput")    # .9*norm^2
    sc2 = nc.dram_tensor("sc2", [SHP, 1], f32, kind="ExternalInput")    # .9*norm
    nrm0 = nc.dram_tensor("nrm0", [SHP, 1], f32, kind="ExternalInput")  # norm
    zeros = nc.dram_tensor("zeros", [SHP, C], f32, kind="ExternalInput")
    out = nc.dram_tensor("out", [SHP, C], f32, kind="ExternalOutput")

    BF = bool(os.environ.get("K_BF16"))
    gdt = bf16 if BF else f32
    GW = 128 if BF else C
    h0 = nc.dram_tensor("h0", [SHP, C], f32, kind="Internal")
    h0s = nc.dram_tensor("h0s", [SHP, C], f32, kind="Internal")
    ag_in = nc.dram_tensor("ag_in", [SHP, GW], gdt, kind="Internal")
    gfull = nc.dram_tensor("gfull", [NTAB, GW], gdt, kind="Internal",
                           addr_space="Shared")
    glocal = nc.dram_tensor("glocal", [NTAB, GW], gdt, kind="Internal")
    aggs = [nc.dram_tensor(f"agg{b}", [SHP, C], f32, kind="Internal")
            for b in range(NBUCK)]

    def nrows(t, a, w):  # node-table AP [128, w, C] at tile-col a
        return t[:, :].rearrange("(p a) c -> p a c", p=128)[:, a:a + w, :]

    def grows(t, a, w):  # like nrows but for GW-wide g tables: first C cols
        return t[:, :].rearrange("(p a) c -> p a c", p=128)[:, a:a + w, 0:C]

    def vrows(t, a, w):
        return t[:, :].rearrange("(p a) o -> p a o", p=128)[:, a:a + w, :]

    with tile.TileContext(nc) as tc:
        # ---------------- MLP ----------------
        with tc.tile_pool(name="mwt", bufs=1) as mwt, \
             tc.tile_pool(name="msb", bufs=3) as msb, \
             tc.tile_pool(name="mps", bufs=2, space="PSUM") as mps:
            ident = mwt.tile([128, 128], f32)
            make_identity(nc, ident[:])
            w1t = mwt.tile([128, 4, H], f32)
            nc.sync.dma_start(out=w1t[:], in_=w1[:, :].rearrange("(k p) h -> p k h", p=128))
            w2t = mwt.tile([128, 2, C], f32)
            nc.sync.dma_start(out=w2t[:], in_=w2[:, :].rearrange("(k p) h -> p k h", p=128))

            for t in range(TT):
                x_t = msb.tile([128, F_IN], f32, tag="x")
                nc.sync.dma_start(out=x_t[:], in_=feat[t * 128:(t + 1) * 128, :])
                xT = msb.tile([128, 4, 128], f32, tag="xT")
                for kc in range(4):
                    tp = mps.tile([128, 128], f32, tag="tp")
                    nc.tensor.transpose(out=tp[:], in_=x_t[:, kc * 128:(kc + 1) * 128],
                                        identity=ident[:])
                    nc.vector.tensor_copy(out=xT[:, kc, :], in_=tp[:])
                h1 = msb.tile([128, 2, 128], f32, tag="h1")
                for hh in range(2):
                    p1 = mps.tile([128, 128], f32, tag="p1")
                    for kc in range(4):
                        nc.tensor.matmul(out=p1[:], lhsT=w1t[:, kc, hh * 128:(hh + 1) * 128],
                                         rhs=xT[:, kc, :], start=(kc == 0), stop=(kc == 3))
                    nc.scalar.activation(out=h1[:, hh, :], in_=p1[:], func=Act.Relu)
                p2 = mps.tile([64, 128], f32, tag="p2")
                for kk in range(2):
                    nc.tensor.matmul(out=p2[:], lhsT=w2t[:, kk, :], rhs=h1[:, kk, :],
                                     start=(kk == 0), stop=(kk == 1))
                h2s = msb.tile([64, 128], f32, tag="h2s")
                nc.vector.tensor_copy(out=h2s[:], in_=p2[:])
                tp2 = mps.tile([128, 64], f32, tag="tp2")
                nc.tensor.transpose(out=tp2[:], in_=h2s[:], identity=ident[:64, :64])
                h0t = msb.tile([128, C], f32, tag="h0t")
                nc.vector.tensor_copy(out=h0t[:], in_=tp2[:])
                nc.sync.dma_start(out=h0[t * 128:(t + 1) * 128, :], in_=h0t[:])

        with tc.tile_pool(name="cst", bufs=1) as cst, \
             tc.tile_pool(name="usb", bufs=2) as usb, \
             tc.tile_pool(name="gp", bufs=3) as gp, \
             tc.tile_pool(name="bp", bufs=3) as bp, \
             tc.tile_pool(name="ip", bufs=3) as ip, \
             tc.tile_pool(name="st", bufs=3) as stp, \
             tc.tile_pool(name="pp", bufs=8, space="PSUM") as pp:
            nrm1_t = cst.tile([128, TT, 1], f32)
            nc.sync.dma_start(out=nrm1_t[:], in_=vrows(nrm1, 0, TT))
            sc1_t = cst.tile([128, TT, 1], f32)
            nc.sync.dma_start(out=sc1_t[:], in_=vrows(sc1, 0, TT))
            sc2_t = cst.tile([128, TT, 1], f32)
            nc.sync.dma_start(out=sc2_t[:], in_=vrows(sc2, 0, TT))
            nrm0_t = cst.tile([128, TT, 1], f32)
            nc.sync.dma_start(out=nrm0_t[:], in_=vrows(nrm0, 0, TT))

            # h0s = alpha*norm*h0 ; g0 = norm*h0 -> ag_in
            for a in range(0, TT, UPD):
                h0_t = usb.tile([128, UPD, C], f32, tag="u0")
                nc.sync.dma_start(out=h0_t[:], in_=nrows(h0, a, UPD))
                t1 = usb.tile([128, UPD, C], f32, tag="u1")
                nc.vector.tensor_tensor(out=t1[:], in0=h0_t[:],
                                        in1=nrm1_t[:, a:a + UPD, :].to_broadcast([128, UPD, C]),
                                        op=Alu.mult)
                nc.sync.dma_start(out=nrows(h0s, a, UPD), in_=t1[:])
                t2 = usb.tile([128, UPD, C], gdt, tag="u2")
                nc.vector.tensor_tensor(out=t2[:], in0=h0_t[:],
                                        in1=nrm0_t[:, a:a + UPD, :].to_broadcast([128, UPD, C]),
                                        op=Alu.mult)
                nc.sync.dma_start(out=grows(ag_in, a, UPD), in_=t2[:])

            # ---------------- propagation ----------------
            for it in range(k_iter):
                nc.gpsimd.collective_compute(
                    "AllGather", Alu.bypass,
                    replica_groups=[list(range(N_CORES))],
                    ins=[ag_in[:, :].opt()], outs=[gfull[:, :].opt()],
                )
                nc.sync.dma_start(out=glocal[:, :], in_=gfull[:, :])
                for b in range(NBUCK):
                    nc.sync.dma_start(out=aggs[b][:, :], in_=zeros[:, :])

                gi = 0
                for b in range(NBUCK):
                    for _gg in range(GB[b]):
                        g0c = gi * GROUP
                        gix = ip.tile([128, GROUP * 8], i16, tag="gix")
                        nc.sync.dma_start(out=gix[:], in_=gidx[:, g0c * 8:(g0c + GROUP) * 8])
                        if BF:
                            m_b = bp.tile([128, GROUP, C], bf16, tag="mb")
                            _dma_gather_raw(nc, m_b[:],
                                            glocal[b * BUCK:(b + 1) * BUCK, 0:C],
                                            gix[:], GROUP * 128, C, GW)
                        else:
                            m_f = gp.tile([128, GROUP, C], f32, tag="mf")
                            nc.gpsimd.dma_gather(
                                out_ap=m_f[:], in_ap=glocal[b * BUCK:(b + 1) * BUCK, :],
                                idxs_ap=gix[:], num_idxs=GROUP * 128,
                                num_idxs_reg=GROUP * 128, elem_size=C)
                            m_b = bp.tile([128, GROUP, C], bf16, tag="mb")
                            if gi % 2 == 0:
                                nc.vector.tensor_copy(out=m_b[:], in_=m_f[:])
                            else:
                                nc.scalar.activation(out=m_b[:], in_=m_f[:], func=Act.Copy)
                        oh_t = ip.tile([128, GROUP, SLOTS], bf16, tag="oh")
                        nc.sync.dma_start(out=oh_t[:], in_=oneh[:, g0c:g0c + GROUP, :])
                        six = ip.tile([128, GROUP * 2], i16, tag="six")
                        nc.sync.dma_start(out=six[:], in_=sidx[:, g0c * 2:(g0c + GROUP) * 2])

                        stg = stp.tile([128, GROUP // 4, C], f32, tag="stg")
                        for pt in range(GROUP // 4):
                            ps = pp.tile([128, C], f32, tag="ps")
                            for q in range(4):
                                cc = pt * 4 + q
                                nc.tensor.matmul(
                                    out=ps[q * 32:q * 32 + 32, :],
                                    lhsT=oh_t[:, cc, :], rhs=m_b[:, cc, :],
                                    start=True, stop=True,
                                    tile_position=(0, q * 32))
                            if pt % 2 == 0:
                                nc.vector.tensor_copy(out=stg[:, pt, :], in_=ps[:])
                            else:
                                nc.scalar.activation(out=stg[:, pt, :], in_=ps[:], func=Act.Copy)
                        if os.environ.get("K_NO_SCATTER"):
                            nc.sync.dma_start(out=aggs[b][gi % 4 * 1024:gi % 4 * 1024 + GROUP * 32, :]
                                              .rearrange("(a p) c -> p a c", p=128), in_=stg[:])
                        else:
                            nc.gpsimd.dma_scatter_add(
                                out_ap=aggs[b][:, :], in_ap=stg[:],
                                idxs_ap=six[:], num_idxs=GROUP * 32,
                                num_idxs_reg=GROUP * 32, elem_size=C)
                        gi += 1

                # update / final
                for a in range(0, TT, UPD):
                    a0 = usb.tile([128, UPD, C], f32, tag="a0")
                    nc.sync.dma_start(out=a0[:], in_=nrows(aggs[0], a, UPD))
                    a1 = usb.tile([128, UPD, C], f32, tag="a1")
                    nc.sync.dma_start(out=a1[:], in_=nrows(aggs[1], a, UPD))
                    nc.vector.tensor_tensor(out=a0[:], in0=a0[:], in1=a1[:], op=Alu.add)
                    a2 = usb.tile([128, UPD, C], f32, tag="a2")
                    nc.sync.dma_start(out=a2[:], in_=nrows(aggs[2], a, UPD))
                    a3 = usb.tile([128, UPD, C], f32, tag="a3")
                    nc.sync.dma_start(out=a3[:], in_=nrows(aggs[3], a, UPD))
                    nc.vector.tensor_tensor(out=a2[:], in0=a2[:], in1=a3[:], op=Alu.add)
                    nc.vector.tensor_tensor(out=a0[:], in0=a0[:], in1=a2[:], op=Alu.add)

                    if it < k_iter - 1:
                        hs = usb.tile([128, UPD, C], f32, tag="hs")
                        nc.sync.dma_start(out=hs[:], in_=nrows(h0s, a, UPD))
                        nc.vector.tensor_tensor(out=a0[:], in0=a0[:],
                                                in1=sc1_t[:, a:a + UPD, :].to_broadcast([128, UPD, C]),
                                                op=Alu.mult)
                        gn = usb.tile([128, UPD, C], gdt, tag="gn")
                        nc.vector.tensor_tensor(out=gn[:], in0=a0[:], in1=hs[:], op=Alu.add)
                        nc.sync.dma_start(out=grows(ag_in, a, UPD), in_=gn[:])
                    else:
                        hh_ = usb.tile([128, UPD, C], f32, tag="hh")
                        nc.sync.dma_start(out=hh_[:], in_=nrows(h0, a, UPD))
                        nc.vector.tensor_tensor(out=a0[:], in0=a0[:],
                                                in1=sc2_t[:, a:a + UPD, :].to_broadcast([128, UPD, C]),
                                                op=Alu.mult)
                        nc.vector.tensor_scalar_mul(out=hh_[:], in0=hh_[:], scalar1=ALPHA)
                        nc.vector.tensor_tensor(out=a0[:], in0=a0[:], in1=hh_[:], op=Alu.add)
                        mx = usb.tile([128, UPD, 1], f32, tag="mx")
                        nc.vector.tensor_reduce(out=mx[:], in_=a0[:],
                                                axis=mybir.AxisListType.X, op=Alu.max)
                        nc.vector.tensor_tensor(out=a0[:], in0=a0[:],
                                                in1=mx[:].to_broadcast([128, UPD, C]),
                                                op=Alu.subtract)
                        ex = usb.tile([128, UPD, C], f32, tag="ex")
                        nc.scalar.activation(out=ex[:], in_=a0[:], func=Act.Exp)
                        sm = usb.tile([128, UPD, 1], f32, tag="sm")
                        nc.vector.tensor_reduce(out=sm[:], in_=ex[:],
                                                axis=mybir.AxisListType.X, op=Alu.add)
                        ls = usb.tile([128, UPD, 1], f32, tag="ls")
                        nc.scalar.activation(out=ls[:], in_=sm[:], func=Act.Ln)
                        nc.vector.tensor_tensor(out=a0[:], in0=a0[:],
                                                in1=ls[:].to_broadcast([128, UPD, C]),
                                                op=Alu.subtract)
                        nc.sync.dma_start(out=nrows(out, a, UPD), in_=a0[:])

    nc.compile()
    return nc


# ----------------------------------------------------------------------------
# entry point
# ----------------------------------------------------------------------------

def kernel(feat, w1, b1, w2, b2, src, dst, k_iter=K_ITER):
    feat = np.asarray(feat, np.float32)
    w1 = np.asarray(w1, np.float32)
    w2 = np.asarray(w2, np.float32)

    norm, per_core = _preprocess(np.asarray(src), np.asarray(dst))
    GB = []
    for b in range(NBUCK):
        mx = max((len(pc[b][0]) + GROUP - 1) // GROUP for pc in per_core)
        GB.append(max(mx, 1))
    nc = _build_nc(GB, k_iter)

    rowperm = np.empty(SHP, np.int64)   # device row -> local node
    for n_ in range(SHP):
        rowperm[_perm(n_)] = n_

    in_maps = []
    for c in range(N_CORES):
        gidx_a, oneh_a, sidx_a = _emit_core_arrays(per_core[c], GB)
        loc = np.zeros(SHP, np.float32)
        loc[:SH] = norm[c * SH:(c + 1) * SH]
        fpad = np.zeros((SHP, F_IN), np.float32)
        fpad[:SH] = feat[c * SH:(c + 1) * SH]
        nl = loc[rowperm][:, None]          # permuted norm column
        in_maps.append({
            "feat": fpad[rowperm], "w1": w1, "w2": w2,
            "gidx": gidx_a, "oneh": oneh_a, "sidx": sidx_a,
            "nrm1": (ALPHA * nl).astype(np.float32),
            "sc1": (0.9 * nl * nl).astype(np.float32),
            "sc2": (0.9 * nl).astype(np.float32),
            "nrm0": nl.astype(np.float32),
            "zeros": np.zeros((SHP, C), np.float32),
        })

    res = run_bass_kernel_spmd(nc, in_maps, core_ids=list(range(N_CORES)),
                               trace=bool(os.environ.get("K_TRACE")))
    if res.exec_time_ns is not None:
        print(f"HW exec time: {res.exec_time_ns} ns")
    parts = []
    for c in range(N_CORES):
        dev = res.results[c]["out"]          # [SHP, C], device rows
        node_major = dev[[_perm(n_) for n_ in range(SH)]]
        parts.append(node_major)
    return np.concatenate(parts, axis=0).astype(np.float32)


# revision 11
# speedup vs baseline: 7.0379x; 7.0379x over previous
"""APPNP GNN kernel for 8 Trainium2 NeuronCores.

Distribution: nodes (feat rows / output rows) sharded 12500/core (padded to
12544). Each core owns the edges into its dst shard. Per APPNP step the
normalized state g = h*norm is AllGathered (the halo exchange), then each core:
  - ant dma_gather pulls one f32 row per edge (edges pre-grouped on the host
    into 4 src-buckets so int16 indices address <=25088-row table windows,
    dst-sorted within bucket, packed into 128-edge chunks of whole segments),
  - a per-chunk one-hot matmul on the TensorEngine pre-accumulates the segment
    sums into 32-slot PSUM groups (bf16 operands, f32 accumulation),
  - ant dma_scatter_add lands the (unique-dst) slot rows into a per-bucket
    aggregate table (pad slots go to a dump row),
  - the update g' = 0.9*norm^2*(sum of bucket tables) + 0.1*norm*h0 is fused
    on the VectorEngine; the final step applies log_softmax instead.
The 2-layer MLP encoder runs on-device before the propagation.

Device tables use a transposed row convention r = (n%128)*98 + n//128 so every
node-table DMA is one contiguous 25KB descriptor per partition; the host
permutes feat/norm inputs and inverse-permutes the output.
"""
import sys, os
sys.path.insert(0, "/opt/trn_rl_repo")
import numpy as np
import ml_dtypes

import concourse.bass as bass
import concourse.tile as tile
from concourse import bacc, mybir
from concourse.bass_utils import run_bass_kernel_spmd
from concourse.masks import make_identity

bf16 = mybir.dt.bfloat16
f32 = mybir.dt.float32
i16 = mybir.dt.int16
Alu = mybir.AluOpType
Act = mybir.ActivationFunctionType

N = 100000
F_IN = 512
H = 256
C = 64
K_ITER = 10
ALPHA = 0.1
N_CORES = 8
SH = 12500
TT = 98
SHP = 12544             # 128*98
NTAB = SHP * N_CORES    # 100352
NBUCK = 4
BUCK = NTAB // NBUCK    # 25088
GROUP = int(os.environ.get("K_GROUP", "32"))  # chunks per gather/scatter call
SLOTS = 32
DUMP = 12543            # perm(12543) == 12543 (pad node)
UPD = 7                 # node-tile cols per update op (98 = 14*7)


def _perm(n):
    return (n % 128) * TT + n // 128


# ----------------------------------------------------------------------------
# host-side graph preprocessing
# ----------------------------------------------------------------------------

def _build_core_tables(table_row, dst_row):
    """table_row: gather-table row per edge (already permuted+core-offset).
    dst_row: permuted local dst row per edge.
    Returns per-bucket packed chunk tables."""
    per_bucket = []
    for b in range(NBUCK):
        sel = (table_row >= b * BUCK) & (table_row < (b + 1) * BUCK)
        d = dst_row[sel]
        r = (table_row[sel] - b * BUCK).astype(np.int32)
        order = np.argsort(d, kind="stable")
        d = d[order]; r = r[order]
        ne = len(d)
        chunks_b = []
        if ne:
            seg_start = np.flatnonzero(np.r_[True, d[1:] != d[:-1]])
            seg_end = np.r_[seg_start[1:], ne]
            seg_len = seg_end - seg_start
            seg_dst = d[seg_start]
            assert seg_len.max() <= 128
            cur_edges = []; cur_dst = []; cur_n = 0
            for s in range(len(seg_start)):
                L = seg_len[s]
                if cur_dst and (cur_n + L > 128 or len(cur_dst) >= SLOTS):
                    chunks_b.append((cur_edges, cur_dst))
                    cur_edges = []; cur_dst = []; cur_n = 0
                cur_edges.append((seg_start[s], seg_end[s]))
                cur_dst.append(seg_dst[s])
                cur_n += L
            if cur_dst:
                chunks_b.append((cur_edges, cur_dst))
        per_bucket.append((chunks_b, r))
    return per_bucket


def _emit_core_arrays(per_bucket, GB):
    """Emit device tables for one core padded to the global schedule GB."""
    NCH = sum(GB) * GROUP
    gidx = np.zeros((16, NCH * 8), np.int16)
    oneh = np.zeros((128, NCH, SLOTS), ml_dtypes.bfloat16)
    sidx = np.full((16, NCH * 2), DUMP, np.int16)
    ci = 0
    for b in range(NBUCK):
        chunks_b, r = per_bucket[b]
        for k in range(GB[b] * GROUP):
            if k < len(chunks_b):
                segs, dsts = chunks_b[k]
                idx16 = np.zeros(128, np.int16)
                oh = np.zeros((128, SLOTS), np.float32)
                sd = np.full(SLOTS, DUMP, np.int32)
                pos = 0
                for j, ((a, e_), dd) in enumerate(zip(segs, dsts)):
                    L = e_ - a
                    idx16[pos:pos + L] = r[a:e_]
                    oh[pos:pos + L, j] = 1.0
                    sd[j] = dd
                    pos += L
                gidx[:, ci * 8:(ci + 1) * 8] = idx16.reshape(8, 16).T
                oneh[:, ci, :] = oh.astype(ml_dtypes.bfloat16)
                sidx[:, ci * 2:(ci + 1) * 2] = sd.astype(np.int16).reshape(2, 16).T
            ci += 1
    return np.tile(gidx, (8, 1)), oneh, np.tile(sidx, (8, 1))


def _preprocess(src, dst):
    loop = np.arange(N, dtype=np.int64)
    src_sl = np.concatenate([np.asarray(src, np.int64), loop])
    dst_sl = np.concatenate([np.asarray(dst, np.int64), loop])
    deg = np.bincount(dst_sl, minlength=N).astype(np.float64)
    norm = (1.0 / np.sqrt(deg)).astype(np.float32)

    src_local = src_sl % SH
    table_row = (src_sl // SH) * SHP + (src_local % 128) * TT + src_local // 128
    dst_core = dst_sl // SH
    dl = dst_sl % SH
    dst_row = ((dl % 128) * TT + dl // 128).astype(np.int32)

    per_core = []
    for c in range(N_CORES):
        m = dst_core == c
        per_core.append(_build_core_tables(table_row[m], dst_row[m]))
    return norm, per_core


# ----------------------------------------------------------------------------
# device kernel
# ----------------------------------------------------------------------------


def _dma_gather_raw(nc, out_ap, in_ap, idxs_ap, num_idxs, elem_size, elem_step, queue_num=0):
    """bass.dma_gather minus the elem_size%256 assert (payload 128B, stride 256B)."""
    import concourse.ap_utils as ap_utils
    from concourse.bass import exact_div
    eng = nc.gpsimd
    assert idxs_ap.dtype == mybir.dt.int16
    assert in_ap.dtype == out_ap.dtype
    assert ap_utils.ap_is_contiguous(out_ap.ap[1:])
    assert ap_utils.ap_is_contiguous(idxs_ap.ap[1:])
    assert in_ap.ap[-1][1] == out_ap.ap[-1][1] == elem_size
    assert out_ap.ap[0][1] * out_ap.ap[1][1] == num_idxs
    assert in_ap.ap[0][0] == elem_step
    stride_bytes = elem_step * mybir.dt.size(in_ap.dtype)
    stride_bytes_256 = exact_div(stride_bytes, 256)
    _in_ap = eng.lower_ap_dma(in_ap, for_custom_bir_dma=True)
    _idxs_ap = eng.lower_ap(idxs_ap)
    _out_ap = eng.lower_ap(out_ap)
    inst = eng.add_instruction(
        mybir.InstDMAGatherAnt(
            name=nc.get_next_instruction_name(),
            ins=[*_in_ap, _idxs_ap, eng.lower_val_access(eng.to_reg(num_idxs))],
            outs=[_out_ap],
            transpose=False,
            num_idxs=num_idxs,
            elem_size=elem_size,
            stride_bytes_256=stride_bytes_256,
            gen_mode=0,
            single_packet=True,
            queue_num=queue_num,
            sbuf_tokens_per_rank=0,
            sbuf_free_dim_per_rank=0,
            sbuf_free_dim_pad_per_rank=0,
            sbuf_byte_offset=0,
        )
    )
    return inst

def _build_nc(GB, k_iter):
    NCH = sum(GB) * GROUP
    nc = bacc.Bacc("TRN2", target_bir_lowering=False, debug=False,
                   num_devices=N_CORES, num_swdge_queues=4)
    feat = nc.dram_tensor("feat", [SHP, F_IN], f32, kind="ExternalInput")
    w1 = nc.dram_tensor("w1", [F_IN, H], f32, kind="ExternalInput")
    w2 = nc.dram_tensor("w2", [H, C], f32, kind="ExternalInput")
    gidx = nc.dram_tensor("gidx", [128, NCH * 8], i16, kind="ExternalInput")
    oneh = nc.dram_tensor("oneh", [128, NCH, SLOTS], bf16, kind="ExternalInput")
    sidx = nc.dram_tensor("sidx", [128, NCH * 2], i16, kind="ExternalInput")
    nrm1 = nc.dram_tensor("nrm1", [SHP, 1], f32, kind="ExternalInput")  # alpha*norm
    sc1 = nc.dram_tensor("sc1", [SHP, 1], f32, kind="ExternalInput")    # .9*norm^2
    sc2 = nc.dram_tensor("sc2", [SHP, 1], f32, kind="ExternalInput")    # .9*norm
    nrm0 = nc.dram_tensor("nrm0", [SHP, 1], f32, kind="ExternalInput")  # norm
    zeros = nc.dram_tensor("zeros", [SHP, C], f32, kind="ExternalInput")
    out = nc.dram_tensor("out", [SHP, C], f32, kind="ExternalOutput")

    BF = bool(os.environ.get("K_BF16"))
    gdt = bf16 if BF else f32
    GW = 128 if BF else C
    h0 = nc.dram_tensor("h0", [SHP, C], f32, kind="Internal")
    h0s = nc.dram_tensor("h0s", [SHP, C], f32, kind="Internal")
    ag_in = nc.dram_tensor("ag_in", [SHP, GW], gdt, kind="Internal")
    gfull = nc.dram_tensor("gfull", [NTAB, GW], gdt, kind="Internal",
                           addr_space="Shared")
    glocal = nc.dram_tensor("glocal", [NTAB, GW], gdt, kind="Internal")
    aggs = [nc.dram_tensor(f"agg{b}", [SHP, C], f32, kind="Internal")
            for b in range(NBUCK)]

    def nrows(t, a, w):  # node-table AP [128, w, C] at tile-col a
        return t[:, :].rearrange("(p a) c -> p a c", p=128)[:, a:a + w, :]

    def grows(t, a, w):  # like nrows but for GW-wide g tables: first C cols
        return t[:, :].rearrange("(p a) c -> p a c", p=128)[:, a:a + w, 0:C]

    def vrows(t, a, w):
        return t[:, :].rearrange("(p a) o -> p a o", p=128)[:, a:a + w, :]

    with tile.TileContext(nc) as tc:
        # ---------------- MLP ----------------
        with tc.tile_pool(name="mwt", bufs=1) as mwt, \
             tc.tile_pool(name="msb", bufs=3) as msb, \
             tc.tile_pool(name="mps", bufs=2, space="PSUM") as mps:
            ident = mwt.tile([128, 128], f32)
            make_identity(nc, ident[:])
            w1t = mwt.tile([128, 4, H], f32)
            nc.sync.dma_start(out=w1t[:], in_=w1[:, :].rearrange("(k p) h -> p k h", p=128))
            w2t = mwt.tile([128, 2, C], f32)
            nc.sync.dma_start(out=w2t[:], in_=w2[:, :].rearrange("(k p) h -> p k h", p=128))

            for t in range(TT):
                x_t = msb.tile([128, F_IN], f32, tag="x")
                nc.sync.dma_start(out=x_t[:], in_=feat[t * 128:(t + 1) * 128, :])
                xT = msb.tile([128, 4, 128], f32, tag="xT")
                for kc in range(4):
                    tp = mps.tile([128, 128], f32, tag="tp")
                    nc.tensor.transpose(out=tp[:], in_=x_t[:, kc * 128:(kc + 1) * 128],
                                        identity=ident[:])
                    nc.vector.tensor_copy(out=xT[:, kc, :], in_=tp[:])
                h1 = msb.tile([128, 2, 128], f32, tag="h1")
                for hh in range(2):
                    p1 = mps.tile([128, 128], f32, tag="p1")
                    for kc in range(4):
                        nc.tensor.matmul(out=p1[:], lhsT=w1t[:, kc, hh * 128:(hh + 1) * 128],
                                         rhs=xT[:, kc, :], start=(kc == 0), stop=(kc == 3))
                    nc.scalar.activation(out=h1[:, hh, :], in_=p1[:], func=Act.Relu)
                p2 = mps.tile([64, 128], f32, tag="p2")
                for kk in range(2):
                    nc.tensor.matmul(out=p2[:], lhsT=w2t[:, kk, :], rhs=h1[:, kk, :],
                                     start=(kk == 0), stop=(kk == 1))
                h2s = msb.tile([64, 128], f32, tag="h2s")
                nc.vector.tensor_copy(out=h2s[:], in_=p2[:])
                tp2 = mps.tile([128, 64], f32, tag="tp2")
                nc.tensor.transpose(out=tp2[:], in_=h2s[:], identity=ident[:64, :64])
                h0t = msb.tile([128, C], f32, tag="h0t")
                nc.vector.tensor_copy(out=h0t[:], in_=tp2[:])
                nc.sync.dma_start(out=h0[t * 128:(t + 1) * 128, :], in_=h0t[:])

        with tc.tile_pool(name="cst", bufs=1) as cst, \
             tc.tile_pool(name="usb", bufs=2) as usb, \
             tc.tile_pool(name="gp", bufs=3) as gp, \
             tc.tile_pool(name="bp", bufs=3) as bp, \
             tc.tile_pool(name="ip", bufs=3) as ip, \
             tc.tile_pool(name="st", bufs=3) as stp, \
             tc.tile_pool(name="pp", bufs=8, space="PSUM") as pp:
            nrm1_t = cst.tile([128, TT, 1], f32)
            nc.sync.dma_start(out=nrm1_t[:], in_=vrows(nrm1, 0, TT))
            sc1_t = cst.tile([128, TT, 1], f32)
            nc.sync.dma_start(out=sc1_t[:], in_=vrows(sc1, 0, TT))
            sc2_t = cst.tile([128, TT, 1], f32)
            nc.sync.dma_start(out=sc2_t[:], in_=vrows(sc2, 0, TT))
            nrm0_t = cst.tile([128, TT, 1], f32)
            nc.sync.dma_start(out=nrm0_t[:], in_=vrows(nrm0, 0, TT))

            # h0s = alpha*norm*h0 ; g0 = norm*h0 -> ag_in
            for a in range(0, TT, UPD):
                h0_t = usb.tile([128, UPD, C], f32, tag="u0")
                nc.sync.dma_start(out=h0_t[:], in_=nrows(h0, a, UPD))
                t1 = usb.tile([128, UPD, C], f32, tag="u1")
                nc.vector.tensor_tensor(out=t1[:], in0=h0_t[:],
                                        in1=nrm1_t[:, a:a + UPD, :].to_broadcast([128, UPD, C]),
                                        op=Alu.mult)
                nc.sync.dma_start(out=nrows(h0s, a, UPD), in_=t1[:])
                t2 = usb.tile([128, UPD, C], gdt, tag="u2")
                nc.vector.tensor_tensor(out=t2[:], in0=h0_t[:],
                                        in1=nrm0_t[:, a:a + UPD, :].to_broadcast([128, UPD, C]),
                                        op=Alu.mult)
                nc.sync.dma_start(out=grows(ag_in, a, UPD), in_=t2[:])

            # ---------------- propagation ----------------
            for it in range(k_iter):
                nc.gpsimd.collective_compute(
                    "AllGather", Alu.bypass,
                    replica_groups=[list(range(N_CORES))],
                    ins=[ag_in[:, :].opt()], outs=[gfull[:, :].opt()],
                )
                nc.sync.dma_start(out=glocal[:, :], in_=gfull[:, :])
                for b in range(NBUCK):
                    nc.sync.dma_start(out=aggs[b][:, :], in_=zeros[:, :])

                gi = 0
                for b in range(NBUCK):
                    for _gg in range(GB[b]):
                        g0c = gi * GROUP
                        gix = ip.tile([128, GROUP * 8], i16, tag="gix")
                        nc.sync.dma_start(out=gix[:], in_=gidx[:, g0c * 8:(g0c + GROUP) * 8])
                        if BF:
                            m_b = bp.tile([128, GROUP, C], bf16, tag="mb")
                            _dma_gather_raw(nc, m_b[:],
                                            glocal[b * BUCK:(b + 1) * BUCK, 0:C],
                                            gix[:], GROUP * 128, C, GW,
                                            queue_num=gi % 4)
                        else:
                            m_f = gp.tile([128, GROUP, C], f32, tag="mf")
                            nc.gpsimd.dma_gather(
                                out_ap=m_f[:], in_ap=glocal[b * BUCK:(b + 1) * BUCK, :],
                                idxs_ap=gix[:], num_idxs=GROUP * 128,
                                num_idxs_reg=GROUP * 128, elem_size=C,
                                queue_num=gi % 4)
                            m_b = bp.tile([128, GROUP, C], bf16, tag="mb")
                            if gi % 2 == 0:
                                nc.vector.tensor_copy(out=m_b[:], in_=m_f[:])
                            else:
                                nc.scalar.activation(out=m_b[:], in_=m_f[:], func=Act.Copy)
                        oh_t = ip.tile([128, GROUP, SLOTS], bf16, tag="oh")
                        nc.sync.dma_start(out=oh_t[:], in_=oneh[:, g0c:g0c + GROUP, :])
                        six = ip.tile([128, GROUP * 2], i16, tag="six")
                        nc.sync.dma_start(out=six[:], in_=sidx[:, g0c * 2:(g0c + GROUP) * 2])

                        stg = stp.tile([128, GROUP // 4, C], f32, tag="stg")
                        for pt in range(GROUP // 4):
                            ps = pp.tile([128, C], f32, tag="ps")
                            for q in range(4):
                                cc = pt * 4 + q
                                nc.tensor.matmul(
                                    out=ps[q * 32:q * 32 + 32, :],
                                    lhsT=oh_t[:, cc, :], rhs=m_b[:, cc, :],
                                    start=True, stop=True,
                                    tile_position=(0, q * 32))
                            if pt % 2 == 0:
                                nc.vector.tensor_copy(out=stg[:, pt, :], in_=ps[:])
                            else:
                                nc.scalar.activation(out=stg[:, pt, :], in_=ps[:], func=Act.Copy)
                        if os.environ.get("K_NO_SCATTER"):
                            nc.sync.dma_start(out=aggs[b][gi % 4 * 1024:gi % 4 * 1024 + GROUP * 32, :]
                                              .rearrange("(a p) c -> p a c", p=128), in_=stg[:])
                        else:
                            nc.gpsimd.dma_scatter_add(
                                out_ap=aggs[b][:, :], in_ap=stg[:],
                                idxs_ap=six[:], num_idxs=GROUP * 32,
                                num_idxs_reg=GROUP * 32, elem_size=C,
                                queue_num=(gi + 2) % 4)
                        gi += 1

                # update / final
                for a in range(0, TT, UPD):
                    a0 = usb.tile([128, UPD, C], f32, tag="a0")
                    nc.sync.dma_start(out=a0[:], in_=nrows(aggs[0], a, UPD))
                    a1 = usb.tile([128, UPD, C], f32, tag="a1")
                    nc.sync.dma_start(out=a1[:], in_=nrows(aggs[1], a, UPD))
                    nc.vector.tensor_tensor(out=a0[:], in0=a0[:], in1=a1[:], op=Alu.add)
                    a2 = usb.tile([128, UPD, C], f32, tag="a2")
                    nc.sync.dma_start(out=a2[:], in_=nrows(aggs[2], a, UPD))
                    a3 = usb.tile([128, UPD, C], f32, tag="a3")
                    nc.sync.dma_start(out=a3[:], in_=nrows(aggs[3], a, UPD))
                    nc.vector.tensor_tensor(out=a2[:], in0=a2[:], in1=a3[:], op=Alu.add)
                    nc.vector.tensor_tensor(out=a0[:], in0=a0[:], in1=a2[:], op=Alu.add)

                    if it < k_iter - 1:
                        hs = usb.tile([128, UPD, C], f32, tag="hs")
                        nc.sync.dma_start(out=hs[:], in_=nrows(h0s, a, UPD))
                        nc.vector.tensor_tensor(out=a0[:], in0=a0[:],
                                                in1=sc1_t[:, a:a + UPD, :].to_broadcast([128, UPD, C]),
                                                op=Alu.mult)
                        gn = usb.tile([128, UPD, C], gdt, tag="gn")
                        nc.vector.tensor_tensor(out=gn[:], in0=a0[:], in1=hs[:], op=Alu.add)
                        nc.sync.dma_start(out=grows(ag_in, a, UPD), in_=gn[:])
                    else:
                        hh_ = usb.tile([128, UPD, C], f32, tag="hh")
                        nc.sync.dma_start(out=hh_[:], in_=nrows(h0, a, UPD))
                        nc.vector.tensor_tensor(out=a0[:], in0=a0[:],
                                                in1=sc2_t[:, a:a + UPD, :].to_broadcast([128, UPD, C]),
                                                op=Alu.mult)
                        nc.vector.tensor_scalar_mul(out=hh_[:], in0=hh_[:], scalar1=ALPHA)
                        nc.vector.tensor_tensor(out=a0[:], in0=a0[:], in1=hh_[:], op=Alu.add)
                        mx = usb.tile([128, UPD, 1], f32, tag="mx")
                        nc.vector.tensor_reduce(out=mx[:], in_=a0[:],
                                                axis=mybir.AxisListType.X, op=Alu.max)
                        nc.vector.tensor_tensor(out=a0[:], in0=a0[:],
                                                in1=mx[:].to_broadcast([128, UPD, C]),
                                                op=Alu.subtract)
                        ex = usb.tile([128, UPD, C], f32, tag="ex")
                        nc.scalar.activation(out=ex[:], in_=a0[:], func=Act.Exp)
                        sm = usb.tile([128, UPD, 1], f32, tag="sm")
                        nc.vector.tensor_reduce(out=sm[:], in_=ex[:],
                                                axis=mybir.AxisListType.X, op=Alu.add)
                        ls = usb.tile([128, UPD, 1], f32, tag="ls")
                        nc.scalar.activation(out=ls[:], in_=sm[:], func=Act.Ln)
                        nc.vector.tensor_tensor(out=a0[:], in0=a0[:],
                                                in1=ls[:].to_broadcast([128, UPD, C]),
                                                op=Alu.subtract)
                        nc.sync.dma_start(out=nrows(out, a, UPD), in_=a0[:])

    nc.compile()
    return nc


# ----------------------------------------------------------------------------
# entry point
# ----------------------------------------------------------------------------

def kernel(feat, w1, b1, w2, b2, src, dst, k_iter=K_ITER):
    feat = np.asarray(feat, np.float32)
    w1 = np.asarray(w1, np.float32)
    w2 = np.asarray(w2, np.float32)

    norm, per_core = _preprocess(np.asarray(src), np.asarray(dst))
    GB = []
    for b in range(NBUCK):
        mx = max((len(pc[b][0]) + GROUP - 1) // GROUP for pc in per_core)
        GB.append(max(mx, 1))
    nc = _build_nc(GB, k_iter)

    rowperm = np.empty(SHP, np.int64)   # device row -> local node
    for n_ in range(SHP):
        rowperm[_perm(n_)] = n_

    in_maps = []
    for c in range(N_CORES):
        gidx_a, oneh_a, sidx_a = _emit_core_arrays(per_core[c], GB)
        loc = np.zeros(SHP, np.float32)
        loc[:SH] = norm[c * SH:(c + 1) * SH]
        fpad = np.zeros((SHP, F_IN), np.float32)
        fpad[:SH] = feat[c * SH:(c + 1) * SH]
        nl = loc[rowperm][:, None]          # permuted norm column
        in_maps.append({
            "feat": fpad[rowperm], "w1": w1, "w2": w2,
            "gidx": gidx_a, "oneh": oneh_a, "sidx": sidx_a,
            "nrm1": (ALPHA * nl).astype(np.float32),
            "sc1": (0.9 * nl * nl).astype(np.float32),
            "sc2": (0.9 * nl).astype(np.float32),
            "nrm0": nl.astype(np.float32),
            "zeros": np.zeros((SHP, C), np.float32),
        })

    res = run_bass_kernel_spmd(nc, in_maps, core_ids=list(range(N_CORES)),
                               trace=bool(os.environ.get("K_TRACE")))
    if res.exec_time_ns is not None:
        print(f"HW exec time: {res.exec_time_ns} ns")
    parts = []
    for c in range(N_CORES):
        dev = res.results[c]["out"]          # [SHP, C], device rows
        node_major = dev[[_perm(n_) for n_ in range(SH)]]
        parts.append(node_major)
    return np.concatenate(parts, axis=0).astype(np.float32)
